# revision 1
# baseline (speedup 1.0000x reference)
"""BinLinear (BatchNorm -> sign-binarize -> scaled binary linear) on 8 TRN2
NeuronCores.

Reference computation (fp32, full batch):
    mean/var over batch axis of x [8192, 4096]
    h  = (x - mean) * rsqrt(var + eps) * gamma + beta          [8192, 4096]
    hb = sign(h)
    out = (hb @ W.T) * alpha[None, :] * mean_abs(h, axis=1)    [8192, 4096]

Distribution: data-parallel over the batch (1024 rows per core). Each core
computes partial per-feature sum / sum-of-squares for its batch shard; a
32 KB AllReduce produces the full-batch BN statistics on every core. W is
pre-scaled by alpha, transposed to [in, out], and cast to bf16 on the host
(replicated to all cores). The matmul runs on the PE in bf16 (hb is exactly
representable); the per-row beta scale is applied in fp32 when draining PSUM.

Device layout: x is fed pre-transposed per shard as xT [4096, 1024] so that
features sit on SBUF partitions. BN stats are then free-axis reductions, the
normalize+binarize is one tensor_scalar + one Sign activation per tile, and
sign(h) lands directly in the [K=in, M=batch] layout the PE needs for its
stationary operand. beta (mean |h| over features) is a partition-axis
reduction done with a ones-vector matmul.
"""

import numpy as np
import ml_dtypes

import concourse.bacc as bacc
import concourse.mybir as mybir
import concourse.tile as tile
from concourse.bass_utils import run_bass_kernel_spmd

dt = mybir.dt
AF = mybir.ActivationFunctionType
ALU = mybir.AluOpType

N_CORES = 8
B, D = 8192, 4096          # batch, features (D_in == D_out == 4096)
BS = B // N_CORES          # 1024 batch rows per core
KT = D // 128              # 32 k-tiles (features / partitions)
EPS = 1e-5

TRACE = False              # set by test.py for profiling runs
LAST_RESULT = None

_nc_cache = None


def _build():
    nc = bacc.Bacc("TRN2", target_bir_lowering=False, debug=False,
                   num_devices=N_CORES)
    xt_d = nc.dram_tensor("xt", [D, BS], dt.float32, kind="ExternalInput").ap()
    wt_d = nc.dram_tensor("wt", [D, D], dt.bfloat16, kind="ExternalInput").ap()
    gb_d = nc.dram_tensor("gb", [128, 2 * KT], dt.float32,
                          kind="ExternalInput").ap()
    out_d = nc.dram_tensor("out", [BS, D], dt.float32,
                           kind="ExternalOutput").ap()

    with tile.TileContext(nc) as tc:
        with (
            tc.tile_pool(name="const", bufs=1) as const,
            tc.tile_pool(name="xs", bufs=3) as xsp,
            tc.tile_pool(name="sq", bufs=2) as sqp,
            tc.tile_pool(name="h", bufs=3) as hp,
            tc.tile_pool(name="habs", bufs=3) as habsp,
            tc.tile_pool(name="hbT", bufs=KT) as hbtp,
            tc.tile_pool(name="wt", bufs=56) as wtp,
            tc.tile_pool(name="yb", bufs=4) as ybp,
            tc.tile_pool(name="ps", bufs=6, space="PSUM") as psp,
            tc.tile_pool(name="dram", bufs=1, space="DRAM") as dram,
        ):
            # ---- constants -------------------------------------------------
            gb_t = const.tile([128, 2 * KT], dt.float32)
            nc.sync.dma_start(gb_t[:], gb_d[:])
            eps_t = const.tile([128, 1], dt.float32)
            nc.vector.memset(eps_t[:], EPS)
            ones = const.tile([128, 1], dt.bfloat16)
            nc.vector.memset(ones[:], 1.0)

            # ---- phase 1: per-shard feature sums ---------------------------
            # Asymmetric split: the first AllReduce covers 20 k-tiles (its
            # stats are ready before the collective bootstrap barrier ends
            # anyway) so the PE has ~30us of matmul work to bridge the second
            # AllReduce. stat columns: [sum h0 | sq h0 | sum h1 | sq h1].
            HKS = [20, KT - 20]
            HBASE = [0, 2 * HKS[0]]
            KBASE = [0, HKS[0]]
            stat = const.tile([128, 2 * KT], dt.float32)
            for t in range(KT):
                half = 0 if t < HKS[0] else 1
                idx = t - KBASE[half]
                xs = xsp.tile([128, BS], dt.float32, name="xs")
                nc.sync.dma_start(xs[:], xt_d[t * 128:(t + 1) * 128, :])
                c = HBASE[half] + idx
                nc.vector.reduce_sum(stat[:, c:c + 1], xs[:],
                                     axis=mybir.AxisListType.X)
                sq = sqp.tile([128, BS], dt.float32, name="sq")
                c = HBASE[half] + HKS[half] + idx
                nc.scalar.activation(sq[:], xs[:], AF.Square,
                                     accum_out=stat[:, c:c + 1])

            # ---- two pipelined AllReduces (one per feature half) so the
            # first half's BN coefficients land while the second reduces ----
            a_t = const.tile([128, KT], dt.float32)
            b_t = const.tile([128, KT], dt.float32)
            for half in range(2):
                HK = HKS[half]
                hs = slice(HBASE[half], HBASE[half] + 2 * HK)
                ks = slice(KBASE[half], KBASE[half] + HK)
                inb = dram.tile([128, 2 * HK], dt.float32, name=f"inb{half}",
                                tag=f"inb{half}")
                outb = dram.tile([128, 2 * HK], dt.float32,
                                 name=f"outb{half}", tag=f"outb{half}")
                nc.scalar.dma_start(inb[:], stat[:, hs])
                nc.gpsimd.collective_compute(
                    "AllReduce", ALU.add,
                    replica_groups=[list(range(N_CORES))],
                    ins=[inb.opt()], outs=[outb.opt()],
                )
                sall = const.tile([128, 2 * HK], dt.float32,
                                  name=f"sall{half}", tag=f"sall{half}")
                nc.scalar.dma_start(sall[:], outb[:])

                # BN coefficients: a = gamma/std, b = beta - mean*a
                mean = const.tile([128, HK], dt.float32, name=f"mean{half}",
                                  tag=f"mean{half}")
                nc.vector.tensor_scalar_mul(mean[:], sall[:, 0:HK], 1.0 / B)
                var = const.tile([128, HK], dt.float32, name=f"var{half}",
                                 tag=f"var{half}")
                nc.vector.tensor_scalar_mul(var[:], sall[:, HK:2 * HK],
                                            1.0 / B)
                msq = const.tile([128, HK], dt.float32, name=f"msq{half}",
                                 tag=f"msq{half}")
                nc.vector.tensor_mul(msq[:], mean[:], mean[:])
                nc.vector.tensor_sub(var[:], var[:], msq[:])
                std = const.tile([128, HK], dt.float32, name=f"std{half}",
                                 tag=f"std{half}")
                nc.scalar.activation(std[:], var[:], AF.Sqrt,
                                     bias=eps_t[:, 0:1], scale=1.0)
                ivs = const.tile([128, HK], dt.float32, name=f"ivs{half}",
                                 tag=f"ivs{half}")
                nc.vector.reciprocal(ivs[:], std[:])
                nc.vector.tensor_mul(a_t[:, ks], ivs[:], gb_t[:, ks])
                nc.vector.tensor_mul(b_t[:, ks], mean[:], a_t[:, ks])
                nc.vector.tensor_sub(
                    b_t[:, ks],
                    gb_t[:, KT + KBASE[half]:KT + KBASE[half] + HK],
                    b_t[:, ks])

            # ---- phase 2: binarize + |h| ----------------------------------
            # hb = Sign(x*a + b) fused on ACT straight from x. fp8 keeps +-1
            # exact at half the SBUF of bf16. |h| = h * hb on DVE. |h|
            # partition sums via ones-matmul into one accumulating PSUM pair.
            beta_ps = psp.tile([1, BS], dt.float32, tag="beta", bufs=1)
            hbT = []
            for t in range(KT):
                xs2 = xsp.tile([128, BS], dt.float32, name="xs2", tag="xs2")
                nc.gpsimd.dma_start(xs2[:], xt_d[t * 128:(t + 1) * 128, :])
                hb = hbtp.tile([128, BS], dt.float8e4, name="hb")
                nc.scalar.activation(hb[:], xs2[:], AF.Sign,
                                     bias=b_t[:, t:t + 1],
                                     scale=a_t[:, t:t + 1])
                hbT.append(hb)
                h = hp.tile([128, BS], dt.float32, name="h")
                nc.vector.tensor_scalar(h[:], xs2[:], a_t[:, t:t + 1],
                                        b_t[:, t:t + 1], ALU.mult, ALU.add)
                habs = habsp.tile([128, BS], dt.bfloat16, name="habs")
                nc.vector.tensor_mul(habs[:], h[:], hb[:])
                for half in range(BS // 512):
                    nc.tensor.matmul(
                        beta_ps[:, half * 512:(half + 1) * 512], ones[:],
                        habs[:, half * 512:(half + 1) * 512],
                        start=(t == 0), stop=(t == KT - 1))

            # ---- beta: scale then transpose [1, BS] -> [128, BS/128] via
            # a DRAM bounce (DRAM-side access patterns are unconstrained) ----
            beta_sb = const.tile([1, BS], dt.float32)
            nc.vector.tensor_scalar_mul(beta_sb[:], beta_ps[:], 1.0 / D)
            bb = dram.tile([1, BS], dt.float32)
            nc.scalar.dma_start(bb[:], beta_sb[:])
            betaT = const.tile([128, BS // 128], dt.float32)
            nc.scalar.dma_start(
                betaT[:], bb.rearrange("o (j p) -> (o p) j", p=128))

            # ---- phase 3: out[bt, oq] = sum_k hbT[k][:, bt].T @ W[k, oq] ---
            for oq in range(D // 512):
                wts = []
                for k in range(KT):
                    wtt = wtp.tile([128, 512], dt.bfloat16, name="wtt")
                    nc.sync.dma_start(
                        wtt[:],
                        wt_d[k * 128:(k + 1) * 128, oq * 512:(oq + 1) * 512])
                    wts.append(wtt)
                for bt in range(BS // 128):
                    pt = psp.tile([128, 512], dt.float32, name="pt", tag="pt")
                    for k in range(KT):
                        nc.tensor.matmul(
                            pt[:], hbT[k][:, bt * 128:(bt + 1) * 128],
                            wts[k][:], start=(k == 0), stop=(k == KT - 1))
                    yb = ybp.tile([128, 512], dt.float32, name="yb")
                    nc.vector.tensor_scalar_mul(yb[:], pt[:],
                                                betaT[:, bt:bt + 1])
                    nc.gpsimd.dma_start(
                        out_d[bt * 128:(bt + 1) * 128,
                              oq * 512:(oq + 1) * 512], yb[:])

    nc.compile()
    return nc


def kernel(x, bn_gamma, bn_beta, W, alpha):
    global _nc_cache, LAST_RESULT
    x = np.ascontiguousarray(x, dtype=np.float32)
    W = np.asarray(W, dtype=np.float32)
    alpha = np.asarray(alpha, dtype=np.float32)

    # host prep: fold alpha into W, transpose to [in, out], cast bf16
    wt = np.ascontiguousarray((W * alpha[:, None]).T).astype(ml_dtypes.bfloat16)
    # gamma/beta in per-partition layout: gb[p, t] = gamma[t*128 + p]
    gb = np.concatenate(
        [np.asarray(bn_gamma, np.float32).reshape(KT, 128).T,
         np.asarray(bn_beta, np.float32).reshape(KT, 128).T], axis=1)
    gb = np.ascontiguousarray(gb)

    if _nc_cache is None:
        _nc_cache = _build()
    nc = _nc_cache

    in_maps = []
    for c in range(N_CORES):
        xT = np.ascontiguousarray(x[c * BS:(c + 1) * BS, :].T)
        in_maps.append({"xt": xT, "wt": wt, "gb": gb})

    res = run_bass_kernel_spmd(nc, in_maps, core_ids=list(range(N_CORES)),
                               trace=TRACE)
    LAST_RESULT = res
    return np.concatenate([res.results[c]["out"] for c in range(N_CORES)],
                          axis=0)



# revision 9
# speedup vs baseline: 1.1188x; 1.1188x over previous
"""BinLinear (BatchNorm -> sign-binarize -> scaled binary linear) on 8 TRN2
NeuronCores.

Reference computation (fp32, full batch):
    mean/var over batch axis of x [8192, 4096]
    h  = (x - mean) * rsqrt(var + eps) * gamma + beta          [8192, 4096]
    hb = sign(h)
    out = (hb @ W.T) * alpha[None, :] * mean_abs(h, axis=1)    [8192, 4096]

Distribution: data-parallel over the batch (1024 rows per core); a ~50 KB
AllReduce (split in two pipelined chunks) produces full-batch BN statistics.

Key device-side structure (per core, xT layout [feat, batch] so features sit
on SBUF partitions):
  phase 1  x tiles stream in and STAY RESIDENT in SBUF; per-feature sum on
           DVE and sum-of-squares on ACT go to two separate stat tiles so the
           two engines pipeline independently.
  AllReduce chunk A (k-tiles 0..19) then chunk B (20..31), overlapped with
           phase-1 tails; BN coefficients a = gamma/std, b = beta - mean*a.
  phase 2  hb = Sign(a*x + b) on ACT straight from the resident x tiles
           (fp8, exact +-1). h and |h| on DVE in bf16; per-row sum |h| via a
           ones-matmul into an accumulating PSUM pair (beta).
  phase 3  out = hb.T @ W with mixed precision: k-tiles 0..11 as 6 fp8
           DoubleRow pair-matmuls (K=256 per instruction, ~1.9x bf16 MACs/s),
           k-tiles 12..31 in bf16. W is pre-scaled by alpha AND by 2^12 on
           the host (fp8 needs the upscale to clear the e4m3 subnormal
           floor); the 2^-12 is folded into the beta scale at PSUM drain.
"""

import numpy as np
import ml_dtypes

import concourse.bacc as bacc
import concourse.mybir as mybir
import concourse.tile as tile
from concourse.bass_utils import run_bass_kernel_spmd

dt = mybir.dt
AF = mybir.ActivationFunctionType
ALU = mybir.AluOpType
PM = mybir.MatmulPerfMode

N_CORES = 8
B, D = 8192, 4096          # batch, features (D_in == D_out == 4096)
BS = B // N_CORES          # 1024 batch rows per core
KT = D // 128              # 32 k-tiles (features / partitions)
EPS = 1e-5

FP8_KT = 12                # k-tiles 0..11 go through fp8 DoubleRow pairs
NPAIR = FP8_KT // 2        # 6 pair tiles
NB16 = KT - FP8_KT         # 20 bf16 k-tiles
W_SCALE = 4096.0           # 2^12 upscale for fp8 W (folded out via beta)

TRACE = False              # set by test.py for profiling runs
LAST_RESULT = None

_nc_cache = None


def _build():
    nc = bacc.Bacc("TRN2", target_bir_lowering=False, debug=False,
                   num_devices=N_CORES)
    xt_d = nc.dram_tensor("xt", [D, BS], dt.float32, kind="ExternalInput").ap()
    w8_d = nc.dram_tensor("w8", [NPAIR * 128, 2 * D], dt.float8e4,
                          kind="ExternalInput").ap()
    w16_d = nc.dram_tensor("w16", [NB16 * 128, D], dt.bfloat16,
                           kind="ExternalInput").ap()
    gb_d = nc.dram_tensor("gb", [128, 2 * KT], dt.float32,
                          kind="ExternalInput").ap()
    out_d = nc.dram_tensor("out", [BS, D], dt.float32,
                           kind="ExternalOutput").ap()

    with tile.TileContext(nc) as tc:
        NRES = 12              # k-tiles 20..31 stay resident in SBUF
        with (
            tc.tile_pool(name="const", bufs=1) as const,
            tc.tile_pool(name="xs", bufs=6) as xsp,
            tc.tile_pool(name="xr", bufs=NRES) as xrp,
            tc.tile_pool(name="xs2", bufs=6) as xs2p,
            tc.tile_pool(name="sqd", bufs=2) as sqdp,
            tc.tile_pool(name="h", bufs=3) as hp,
            tc.tile_pool(name="habs", bufs=3) as habsp,
            tc.tile_pool(name="hbp", bufs=1) as hbpp,
            tc.tile_pool(name="hbs", bufs=1) as hbsp,
            tc.tile_pool(name="w8", bufs=12) as w8p,
            tc.tile_pool(name="w16", bufs=32) as w16p,
            tc.tile_pool(name="yb", bufs=4) as ybp,
            tc.tile_pool(name="ps", bufs=6, space="PSUM") as psp,
            tc.tile_pool(name="dram", bufs=1, space="DRAM") as dram,
        ):
            # ---- constants -------------------------------------------------
            gb_t = const.tile([128, 2 * KT], dt.float32)
            nc.sync.dma_start(gb_t[:], gb_d[:])
            eps_t = const.tile([128, 1], dt.float32)
            nc.vector.memset(eps_t[:], EPS)
            ones = const.tile([128, 1], dt.bfloat16)
            nc.vector.memset(ones[:], 1.0)

            # ---- phase 1: x resident + per-shard feature stats -------------
            # sum on DVE into stat_sum, sum-of-squares on ACT into stat_sq:
            # independent destination tiles keep the two engines pipelined.
            HKS = [20, KT - 20]
            KBASE = [0, 20]
            stat_sum = const.tile([128, KT], dt.float32)
            stat_sq = const.tile([128, KT], dt.float32)
            xres = {}
            for t in range(KT):
                if t < KT - NRES:
                    xs = xsp.tile([128, BS], dt.float32, name="xs")
                else:
                    xs = xrp.tile([128, BS], dt.float32, name="xk")
                    xres[t] = xs
                nc.sync.dma_start(xs[:], xt_d[t * 128:(t + 1) * 128, :])
                nc.vector.reduce_sum(stat_sum[:, t:t + 1], xs[:],
                                     axis=mybir.AxisListType.X)
                sq = sqdp.tile([128, BS], dt.float8e4, name="sq")
                nc.scalar.activation(sq[:], xs[:], AF.Square,
                                     accum_out=stat_sq[:, t:t + 1])

            # ---- two pipelined AllReduces (chunk A: tiles 0..19, B: rest) --
            a_t = const.tile([128, KT], dt.float32)
            b_t = const.tile([128, KT], dt.float32)
            for half in range(2):
                HK = HKS[half]
                k0 = KBASE[half]
                ks = slice(k0, k0 + HK)
                inb = dram.tile([128, 2 * HK], dt.float32, name=f"inb{half}",
                                tag=f"inb{half}")
                outb = dram.tile([128, 2 * HK], dt.float32,
                                 name=f"outb{half}", tag=f"outb{half}")
                nc.scalar.dma_start(inb[:, 0:HK], stat_sum[:, ks])
                nc.scalar.dma_start(inb[:, HK:2 * HK], stat_sq[:, ks])
                nc.gpsimd.collective_compute(
                    "AllReduce", ALU.add,
                    replica_groups=[list(range(N_CORES))],
                    ins=[inb.opt()], outs=[outb.opt()],
                )
                sall = const.tile([128, 2 * HK], dt.float32,
                                  name=f"sall{half}", tag=f"sall{half}")
                nc.scalar.dma_start(sall[:], outb[:])

                # BN coefficients: a = gamma/std, b = beta - mean*a
                mean = const.tile([128, HK], dt.float32, name=f"mean{half}",
                                  tag=f"mean{half}")
                nc.vector.tensor_scalar_mul(mean[:], sall[:, 0:HK], 1.0 / B)
                var = const.tile([128, HK], dt.float32, name=f"var{half}",
                                 tag=f"var{half}")
                nc.vector.tensor_scalar_mul(var[:], sall[:, HK:2 * HK],
                                            1.0 / B)
                msq = const.tile([128, HK], dt.float32, name=f"msq{half}",
                                 tag=f"msq{half}")
                nc.vector.tensor_mul(msq[:], mean[:], mean[:])
                nc.vector.tensor_sub(var[:], var[:], msq[:])
                std = const.tile([128, HK], dt.float32, name=f"std{half}",
                                 tag=f"std{half}")
                nc.scalar.activation(std[:], var[:], AF.Sqrt,
                                     bias=eps_t[:, 0:1], scale=1.0)
                ivs = const.tile([128, HK], dt.float32, name=f"ivs{half}",
                                 tag=f"ivs{half}")
                nc.vector.reciprocal(ivs[:], std[:])
                nc.vector.tensor_mul(a_t[:, ks], ivs[:], gb_t[:, ks])
                nc.vector.tensor_mul(b_t[:, ks], mean[:], a_t[:, ks])
                nc.vector.tensor_sub(
                    b_t[:, ks],
                    gb_t[:, KT + k0:KT + k0 + HK],
                    b_t[:, ks])

            # ---- phase 2: binarize + |h| from the resident x tiles --------
            # hb = Sign(x*a + b) on ACT (fp8, exact). k-tiles 0..11 write the
            # two halves of DoubleRow pair tiles [128, 2, BS]; 12..31 write
            # plain [128, BS] tiles. |h| = (a*x+b)*hb in bf16 on DVE; per-row
            # sums via an accumulating ones-matmul (beta).
            beta_ps = psp.tile([1, BS], dt.float32, tag="beta", bufs=1)
            hbpair = [hbpp.tile([128, 2 * BS], dt.float8e4, name=f"hbp{i}",
                                tag=f"hbp{i}") for i in range(NPAIR)]
            hbsing = [hbsp.tile([128, BS], dt.float8e4, name=f"hbs{i}",
                                tag=f"hbs{i}") for i in range(NB16)]
            for t in range(KT):
                if t in xres:
                    xs = xres[t]
                else:
                    xs = xs2p.tile([128, BS], dt.float32, name="xs2")
                    nc.gpsimd.dma_start(xs[:],
                                        xt_d[t * 128:(t + 1) * 128, :])
                if t < FP8_KT:
                    hb = hbpair[t // 2][:, (t % 2) * BS:(t % 2 + 1) * BS]
                else:
                    hb = hbsing[t - FP8_KT][:]
                nc.scalar.activation(hb, xs[:], AF.Sign,
                                     bias=b_t[:, t:t + 1],
                                     scale=a_t[:, t:t + 1])
                h = hp.tile([128, BS], dt.bfloat16, name="h")
                nc.vector.tensor_scalar(h[:], xs[:], a_t[:, t:t + 1],
                                        b_t[:, t:t + 1], ALU.mult, ALU.add)
                habs = habsp.tile([128, BS], dt.bfloat16, name="habs")
                nc.vector.tensor_mul(habs[:], h[:], hb)
                for half in range(BS // 512):
                    nc.tensor.matmul(
                        beta_ps[:, half * 512:(half + 1) * 512], ones[:],
                        habs[:, half * 512:(half + 1) * 512],
                        start=(t == 0), stop=(t == KT - 1))

            # ---- beta: scale (incl. the 2^-12 W upscale) then transpose
            # [1, BS] -> [128, BS/128] via a DRAM bounce ---------------------
            beta_sb = const.tile([1, BS], dt.float32)
            nc.vector.tensor_scalar_mul(beta_sb[:], beta_ps[:],
                                        1.0 / (D * W_SCALE))
            bb = dram.tile([1, BS], dt.float32)
            nc.scalar.dma_start(bb[:], beta_sb[:])
            betaT = const.tile([128, BS // 128], dt.float32)
            nc.scalar.dma_start(
                betaT[:], bb.rearrange("o (j p) -> (o p) j", p=128))

            # ---- phase 3: out[bt, oq] = sum_k hb[k][:, bt].T @ W[k, oq] ----
            # 6 fp8 DoubleRow pair matmuls (K=256 each) + 20 bf16 matmuls per
            # accumulation group.
            for oq in range(D // 512):
                w8s = []
                for kp in range(NPAIR):
                    w8t = w8p.tile([128, 2 * 512], dt.float8e4, name="w8t")
                    nc.sync.dma_start(
                        w8t[:].rearrange("p (i o) -> p i o", i=2),
                        w8_d[kp * 128:(kp + 1) * 128, :]
                        .rearrange("p (i o) -> p i o", i=2)
                        [:, :, oq * 512:(oq + 1) * 512])
                    w8s.append(w8t)
                w16s = []
                for j in range(NB16):
                    wtt = w16p.tile([128, 512], dt.bfloat16, name="wtt")
                    nc.sync.dma_start(
                        wtt[:],
                        w16_d[j * 128:(j + 1) * 128,
                              oq * 512:(oq + 1) * 512])
                    w16s.append(wtt)
                for bt in range(BS // 128):
                    pt = psp.tile([128, 512], dt.float32, name="pt", tag="pt")
                    for kp in range(NPAIR):
                        nc.tensor.matmul(
                            pt[:],
                            hbpair[kp][:].rearrange("p (i b) -> p i b", i=2)
                            [:, :, bt * 128:(bt + 1) * 128],
                            w8s[kp][:].rearrange("p (i o) -> p i o", i=2),
                            start=(kp == 0), stop=False,
                            perf_mode=PM.DoubleRow)
                    for j in range(NB16):
                        nc.tensor.matmul(
                            pt[:], hbsing[j][:, bt * 128:(bt + 1) * 128],
                            w16s[j][:], start=False, stop=(j == NB16 - 1))
                    yb = ybp.tile([128, 512], dt.float32, name="yb")
                    nc.vector.tensor_scalar_mul(yb[:], pt[:],
                                                betaT[:, bt:bt + 1])
                    nc.gpsimd.dma_start(
                        out_d[bt * 128:(bt + 1) * 128,
                              oq * 512:(oq + 1) * 512], yb[:])

    nc.compile()
    return nc


def kernel(x, bn_gamma, bn_beta, W, alpha):
    global _nc_cache, LAST_RESULT
    x = np.ascontiguousarray(x, dtype=np.float32)
    W = np.asarray(W, dtype=np.float32)
    alpha = np.asarray(alpha, dtype=np.float32)

    # host prep: fold alpha into W, transpose to [in, out], upscale by 2^12
    ws = np.ascontiguousarray((W * alpha[:, None]).T) * np.float32(W_SCALE)
    # fp8 part: k-tiles 0..11 -> pair layout [kp*128+p, i*D+o],
    # value = ws[(2kp + i)*128 + p, o]
    w8 = ws[:FP8_KT * 128].reshape(NPAIR, 2, 128, D).transpose(0, 2, 1, 3)
    w8 = np.clip(w8, -240.0, 240.0).reshape(NPAIR * 128, 2 * D)
    w8 = np.ascontiguousarray(w8).astype(ml_dtypes.float8_e4m3)
    # bf16 part: k-tiles 12..31 (same 2^12 scale -- exact in bf16)
    w16 = np.ascontiguousarray(ws[FP8_KT * 128:]).astype(ml_dtypes.bfloat16)
    # gamma/beta in per-partition layout: gb[p, t] = gamma[t*128 + p]
    gb = np.concatenate(
        [np.asarray(bn_gamma, np.float32).reshape(KT, 128).T,
         np.asarray(bn_beta, np.float32).reshape(KT, 128).T], axis=1)
    gb = np.ascontiguousarray(gb)

    if _nc_cache is None:
        _nc_cache = _build()
    nc = _nc_cache

    in_maps = []
    for c in range(N_CORES):
        xT = np.ascontiguousarray(x[c * BS:(c + 1) * BS, :].T)
        in_maps.append({"xt": xT, "w8": w8, "w16": w16, "gb": gb})

    res = run_bass_kernel_spmd(nc, in_maps, core_ids=list(range(N_CORES)),
                               trace=TRACE)
    LAST_RESULT = res
    return np.concatenate([res.results[c]["out"] for c in range(N_CORES)],
                          axis=0)


# revision 16
# speedup vs baseline: 1.1871x; 1.0611x over previous
"""BinLinear (BatchNorm -> sign-binarize -> scaled binary linear) on 8 TRN2
NeuronCores.

Reference computation (fp32, full batch):
    mean/var over batch axis of x [8192, 4096]
    h  = (x - mean) * rsqrt(var + eps) * gamma + beta          [8192, 4096]
    hb = sign(h)
    out = (hb @ W.T) * alpha[None, :] * mean_abs(h, axis=1)    [8192, 4096]

Distribution: data-parallel over the batch (1024 rows per core); a ~50 KB
AllReduce (split in two pipelined chunks) produces full-batch BN statistics.

Key device-side structure (per core, xT layout [feat, batch] so features sit
on SBUF partitions):
  phase 1  x tiles stream in and STAY RESIDENT in SBUF; per-feature sum on
           DVE and sum-of-squares on ACT go to two separate stat tiles so the
           two engines pipeline independently.
  AllReduce chunk A (k-tiles 0..19) then chunk B (20..31), overlapped with
           phase-1 tails; BN coefficients a = gamma/std, b = beta - mean*a.
  phase 2  hb = Sign(a*x + b) on ACT straight from the resident x tiles
           (fp8, exact +-1). h and |h| on DVE in bf16; per-row sum |h| via a
           ones-matmul into an accumulating PSUM pair (beta).
  phase 3  out = hb.T @ W with mixed precision: k-tiles 0..11 as 6 fp8
           DoubleRow pair-matmuls (K=256 per instruction, ~1.9x bf16 MACs/s),
           k-tiles 12..31 in bf16. W is pre-scaled by alpha AND by 2^12 on
           the host (fp8 needs the upscale to clear the e4m3 subnormal
           floor); the 2^-12 is folded into the beta scale at PSUM drain.
"""

import numpy as np
import ml_dtypes

import concourse.bacc as bacc
import concourse.mybir as mybir
import concourse.tile as tile
from concourse.bass_utils import run_bass_kernel_spmd

dt = mybir.dt
AF = mybir.ActivationFunctionType
ALU = mybir.AluOpType
PM = mybir.MatmulPerfMode

N_CORES = 8
B, D = 8192, 4096          # batch, features (D_in == D_out == 4096)
BS = B // N_CORES          # 1024 batch rows per core
KT = D // 128              # 32 k-tiles (features / partitions)
EPS = 1e-5

FP8_KT = 14                # k-tiles 0..13 go through fp8 DoubleRow pairs
NPAIR = FP8_KT // 2        # 6 pair tiles
NB16 = KT - FP8_KT         # 20 bf16 k-tiles
W_SCALE = 4096.0           # 2^12 upscale for fp8 W (folded out via beta)

TRACE = False              # set by test.py for profiling runs
LAST_RESULT = None

_nc_cache = None


def _build():
    nc = bacc.Bacc("TRN2", target_bir_lowering=False, debug=False,
                   num_devices=N_CORES)
    xt_d = nc.dram_tensor("xt", [D, BS], dt.float32, kind="ExternalInput").ap()
    w8_d = nc.dram_tensor("w8", [NPAIR * 128, 2 * D], dt.float8e4,
                          kind="ExternalInput").ap()
    w16_d = nc.dram_tensor("w16", [NB16 * 128, D], dt.bfloat16,
                           kind="ExternalInput").ap()
    gb_d = nc.dram_tensor("gb", [128, 2 * KT], dt.float32,
                          kind="ExternalInput").ap()
    out_d = nc.dram_tensor("out", [BS, D], dt.float32,
                           kind="ExternalOutput").ap()

    with tile.TileContext(nc) as tc:
        NRES = 12              # k-tiles 20..31 stay resident in SBUF
        with (
            tc.tile_pool(name="const", bufs=1) as const,
            tc.tile_pool(name="xs", bufs=6) as xsp,
            tc.tile_pool(name="xr", bufs=NRES) as xrp,
            tc.tile_pool(name="xs2", bufs=6) as xs2p,
            tc.tile_pool(name="sqd", bufs=2) as sqdp,
            tc.tile_pool(name="h", bufs=3) as hp,
            tc.tile_pool(name="habs", bufs=3) as habsp,
            tc.tile_pool(name="hbp", bufs=1) as hbpp,
            tc.tile_pool(name="hbs", bufs=1) as hbsp,
            tc.tile_pool(name="w8", bufs=12) as w8p,
            tc.tile_pool(name="w16", bufs=32) as w16p,
            tc.tile_pool(name="yb", bufs=4) as ybp,
            tc.tile_pool(name="ps", bufs=6, space="PSUM") as psp,
            tc.tile_pool(name="dram", bufs=1, space="DRAM") as dram,
        ):
            # ---- constants -------------------------------------------------
            gb_t = const.tile([128, 2 * KT], dt.float32)
            nc.sync.dma_start(gb_t[:], gb_d[:])
            eps_t = const.tile([128, 1], dt.float32)
            nc.vector.memset(eps_t[:], EPS)
            ones = const.tile([128, 1], dt.bfloat16)
            nc.vector.memset(ones[:], 1.0)

            # ---- phase 1: x resident + per-shard feature stats -------------
            # sum on DVE into stat_sum, sum-of-squares on ACT into stat_sq:
            # independent destination tiles keep the two engines pipelined.
            HKS = [16, KT - 16]
            KBASE = [0, 16]
            stat_sum = const.tile([128, KT], dt.float32)
            stat_sq = const.tile([128, KT], dt.float32)
            xres = {}
            for t in range(KT):
                if t < KT - NRES:
                    xs = xsp.tile([128, BS], dt.float32, name="xs")
                else:
                    xs = xrp.tile([128, BS], dt.float32, name="xk")
                    xres[t] = xs
                nc.sync.dma_start(xs[:], xt_d[t * 128:(t + 1) * 128, :])
                nc.vector.reduce_sum(stat_sum[:, t:t + 1], xs[:],
                                     axis=mybir.AxisListType.X)
                sq = sqdp.tile([128, BS], dt.float8e4, name="sq")
                nc.scalar.activation(sq[:], xs[:], AF.Square,
                                     accum_out=stat_sq[:, t:t + 1])

            # ---- two pipelined AllReduces (chunk A: tiles 0..19, B: rest) --
            a_t = const.tile([128, KT], dt.float32)
            b_t = const.tile([128, KT], dt.float32)
            for half in range(2):
                HK = HKS[half]
                k0 = KBASE[half]
                ks = slice(k0, k0 + HK)
                inb = dram.tile([128, 2 * HK], dt.float32, name=f"inb{half}",
                                tag=f"inb{half}")
                outb = dram.tile([128, 2 * HK], dt.float32,
                                 name=f"outb{half}", tag=f"outb{half}")
                nc.scalar.dma_start(inb[:, 0:HK], stat_sum[:, ks])
                nc.scalar.dma_start(inb[:, HK:2 * HK], stat_sq[:, ks])
                nc.gpsimd.collective_compute(
                    "AllReduce", ALU.add,
                    replica_groups=[list(range(N_CORES))],
                    ins=[inb.opt()], outs=[outb.opt()],
                )
                sall = const.tile([128, 2 * HK], dt.float32,
                                  name=f"sall{half}", tag=f"sall{half}")
                nc.scalar.dma_start(sall[:], outb[:])

                # BN coefficients: a = gamma/std, b = beta - mean*a
                mean = const.tile([128, HK], dt.float32, name=f"mean{half}",
                                  tag=f"mean{half}")
                nc.vector.tensor_scalar_mul(mean[:], sall[:, 0:HK], 1.0 / B)
                var = const.tile([128, HK], dt.float32, name=f"var{half}",
                                 tag=f"var{half}")
                nc.vector.tensor_scalar_mul(var[:], sall[:, HK:2 * HK],
                                            1.0 / B)
                msq = const.tile([128, HK], dt.float32, name=f"msq{half}",
                                 tag=f"msq{half}")
                nc.vector.tensor_mul(msq[:], mean[:], mean[:])
                nc.vector.tensor_sub(var[:], var[:], msq[:])
                std = const.tile([128, HK], dt.float32, name=f"std{half}",
                                 tag=f"std{half}")
                nc.scalar.activation(std[:], var[:], AF.Sqrt,
                                     bias=eps_t[:, 0:1], scale=1.0)
                ivs = const.tile([128, HK], dt.float32, name=f"ivs{half}",
                                 tag=f"ivs{half}")
                nc.vector.reciprocal(ivs[:], std[:])
                nc.vector.tensor_mul(a_t[:, ks], ivs[:], gb_t[:, ks])
                nc.vector.tensor_mul(b_t[:, ks], mean[:], a_t[:, ks])
                nc.vector.tensor_sub(
                    b_t[:, ks],
                    gb_t[:, KT + k0:KT + k0 + HK],
                    b_t[:, ks])

            # ---- phase 2: binarize + |h| from the resident x tiles --------
            # hb = Sign(x*a + b) on ACT (fp8, exact). k-tiles 0..11 write the
            # two halves of DoubleRow pair tiles [128, 2, BS]; 12..31 write
            # plain [128, BS] tiles. |h| = (a*x+b)*hb in bf16 on DVE; per-row
            # sums via an accumulating ones-matmul (beta).
            beta_ps = psp.tile([1, BS], dt.float32, tag="beta", bufs=1)
            hbpair = [hbpp.tile([128, 2 * BS], dt.float8e4, name=f"hbp{i}",
                                tag=f"hbp{i}") for i in range(NPAIR)]
            hbsing = [hbsp.tile([128, BS], dt.float8e4, name=f"hbs{i}",
                                tag=f"hbs{i}") for i in range(NB16)]
            # tiles 0..ACT_ABS-1: |h| = (a*x+b)*hb on DVE; tiles ACT_ABS..31:
            # |h| = Abs(a*x+b) on ACT (emitted after all the signs so the
            # scheduler keeps the sign chain -- which gates phase 3 -- first).
            ACT_ABS = 22
            xphase2 = []
            for t in range(KT):
                if t in xres:
                    xs = xres[t]
                else:
                    xs = xs2p.tile([128, BS], dt.float32, name="xs2")
                    nc.gpsimd.dma_start(xs[:],
                                        xt_d[t * 128:(t + 1) * 128, :])
                xphase2.append(xs)
                if t < FP8_KT:
                    hb = hbpair[t // 2][:, (t % 2) * BS:(t % 2 + 1) * BS]
                else:
                    hb = hbsing[t - FP8_KT][:]
                nc.scalar.activation(hb, xs[:], AF.Sign,
                                     bias=b_t[:, t:t + 1],
                                     scale=a_t[:, t:t + 1])
                if t < ACT_ABS:
                    h = hp.tile([128, BS], dt.bfloat16, name="h")
                    nc.vector.tensor_scalar(h[:], xs[:], a_t[:, t:t + 1],
                                            b_t[:, t:t + 1], ALU.mult,
                                            ALU.add)
                    habs = habsp.tile([128, BS], dt.bfloat16, name="habs")
                    nc.vector.tensor_mul(habs[:], h[:], hb)
                    for half in range(BS // 512):
                        nc.tensor.matmul(
                            beta_ps[:, half * 512:(half + 1) * 512], ones[:],
                            habs[:, half * 512:(half + 1) * 512],
                            start=(t == 0), stop=False)
            for t in range(ACT_ABS, KT):
                habs = habsp.tile([128, BS], dt.bfloat16, name="habs2",
                                  tag="habs2")
                nc.scalar.activation(habs[:], xphase2[t][:], AF.Abs,
                                     bias=b_t[:, t:t + 1],
                                     scale=a_t[:, t:t + 1])
                for half in range(BS // 512):
                    nc.tensor.matmul(
                        beta_ps[:, half * 512:(half + 1) * 512], ones[:],
                        habs[:, half * 512:(half + 1) * 512],
                        start=False, stop=(t == KT - 1))

            # ---- beta: scale (incl. the 2^-12 W upscale) then transpose
            # [1, BS] -> [128, BS/128] via SBUF->SBUF DMA --------------------
            beta_sb = const.tile([1, BS], dt.float32)
            nc.vector.tensor_scalar_mul(beta_sb[:], beta_ps[:],
                                        1.0 / (D * W_SCALE))
            betaT = const.tile([128, BS // 128], dt.float32)
            nc.scalar.dma_start(
                betaT[:], beta_sb[:].rearrange("o (j p) -> (o p) j", p=128))

            # ---- phase 3: out[bt, oq] = sum_k hb[k][:, bt].T @ W[k, oq] ----
            # 6 fp8 DoubleRow pair matmuls (K=256 each) + 20 bf16 matmuls per
            # accumulation group.
            for oq in range(D // 512):
                w8s = []
                for kp in range(NPAIR):
                    w8t = w8p.tile([128, 2 * 512], dt.float8e4, name="w8t")
                    nc.sync.dma_start(
                        w8t[:].rearrange("p (i o) -> p i o", i=2),
                        w8_d[kp * 128:(kp + 1) * 128, :]
                        .rearrange("p (i o) -> p i o", i=2)
                        [:, :, oq * 512:(oq + 1) * 512])
                    w8s.append(w8t)
                w16s = []
                for j in range(NB16):
                    wtt = w16p.tile([128, 512], dt.bfloat16, name="wtt")
                    nc.sync.dma_start(
                        wtt[:],
                        w16_d[j * 128:(j + 1) * 128,
                              oq * 512:(oq + 1) * 512])
                    w16s.append(wtt)
                for bt in range(BS // 128):
                    pt = psp.tile([128, 512], dt.float32, name="pt", tag="pt")
                    for kp in range(NPAIR):
                        nc.tensor.matmul(
                            pt[:],
                            hbpair[kp][:].rearrange("p (i b) -> p i b", i=2)
                            [:, :, bt * 128:(bt + 1) * 128],
                            w8s[kp][:].rearrange("p (i o) -> p i o", i=2),
                            start=(kp == 0), stop=False,
                            perf_mode=PM.DoubleRow)
                    for j in range(NB16):
                        nc.tensor.matmul(
                            pt[:], hbsing[j][:, bt * 128:(bt + 1) * 128],
                            w16s[j][:], start=False, stop=(j == NB16 - 1))
                    yb = ybp.tile([128, 512], dt.float32, name="yb")
                    nc.vector.tensor_scalar_mul(yb[:], pt[:],
                                                betaT[:, bt:bt + 1])
                    nc.gpsimd.dma_start(
                        out_d[bt * 128:(bt + 1) * 128,
                              oq * 512:(oq + 1) * 512], yb[:])

    nc.compile()
    return nc


def kernel(x, bn_gamma, bn_beta, W, alpha):
    global _nc_cache, LAST_RESULT
    x = np.ascontiguousarray(x, dtype=np.float32)
    W = np.asarray(W, dtype=np.float32)
    alpha = np.asarray(alpha, dtype=np.float32)

    # host prep: fold alpha into W, transpose to [in, out], upscale by 2^12
    ws = np.ascontiguousarray((W * alpha[:, None]).T) * np.float32(W_SCALE)
    # fp8 part: k-tiles 0..11 -> pair layout [kp*128+p, i*D+o],
    # value = ws[(2kp + i)*128 + p, o]
    w8 = ws[:FP8_KT * 128].reshape(NPAIR, 2, 128, D).transpose(0, 2, 1, 3)
    w8 = np.clip(w8, -240.0, 240.0).reshape(NPAIR * 128, 2 * D)
    w8 = np.ascontiguousarray(w8).astype(ml_dtypes.float8_e4m3)
    # bf16 part: k-tiles 12..31 (same 2^12 scale -- exact in bf16)
    w16 = np.ascontiguousarray(ws[FP8_KT * 128:]).astype(ml_dtypes.bfloat16)
    # gamma/beta in per-partition layout: gb[p, t] = gamma[t*128 + p]
    gb = np.concatenate(
        [np.asarray(bn_gamma, np.float32).reshape(KT, 128).T,
         np.asarray(bn_beta, np.float32).reshape(KT, 128).T], axis=1)
    gb = np.ascontiguousarray(gb)

    if _nc_cache is None:
        _nc_cache = _build()
    nc = _nc_cache

    in_maps = []
    for c in range(N_CORES):
        xT = np.ascontiguousarray(x[c * BS:(c + 1) * BS, :].T)
        in_maps.append({"xt": xT, "w8": w8, "w16": w16, "gb": gb})

    res = run_bass_kernel_spmd(nc, in_maps, core_ids=list(range(N_CORES)),
                               trace=TRACE)
    LAST_RESULT = res
    return np.concatenate([res.results[c]["out"] for c in range(N_CORES)],
                          axis=0)
